# revision 17
# baseline (speedup 1.0000x reference)
"""Causal self-attention on 8 TRN2 NeuronCores.

Problem: B=4, T=2048, D=1024, H=16 heads (hd=64).
  qkv = x @ W_qkv + b_qkv ; causal softmax attention ; y @ W_proj + b_proj

Sharding: DP(4 batches) x TP(2 head-groups) = 8 cores.
  Core c handles batch b = c//2, heads g*8..g*8+7 where g = c%2.
  Each core computes qkv for its 8 heads, attention, and the partial
  projection (its 512 head-channels x W_proj rows). A 2-way ReduceScatter
  between the pair (2b, 2b+1) sums the partials; mid-kernel chunks use
  2 RS parts for pipelining, the last chunk uses ONE part (each 2-core
  RS costs ~10us fixed latency and they serialize, so the pure-tail RS
  must be a single op).

Kernel layout (no transposes anywhere):
  - Host passes x^T [D, T] per batch (bf16); x columns 0:512 are DMA'd
    first so the first qkv matmuls unblock early.
  - Q^T, K^T computed as [dg, T] (partition = head feature) via lhsT=Wq.
  - V computed as [T, dg] (natural), stored per head as [V_h | ones64]
    (128-wide stationary) so the PV matmul emits Y^T in PSUM rows 0-63
    and the softmax row-sums replicated across rows 64-127 -> normalize
    is a plain DVE reciprocal + mul, no cross-partition broadcast.
  - S^T = K_h @ Q_h^T per k-tile with causal column trimming; softmax
    without max-subtraction (scores are small), diagonal-block mask via
    -1e5 add before exp.
  - Y^T is exactly the lhsT the proj matmul wants.
  - ReduceScatter writes the external output directly (bf16); the host
    casts to f32.
All matmuls bf16 (f32 PSUM accumulate).

Scheduling: qkv for chunk c+1 and proj for chunk c-1 are generators
yielding after every matmul, woven one-MM-at-a-time into the ACT-paced
attention stream. Chunk boundaries drain only what the next head-pair
actually needs (tracked per-chunk completion counts) instead of a full
barrier, so the PE never sits behind a DVE cast convoy.
"""

import numpy as np
import ml_dtypes

B, T, D = 4, 2048, 1024
H = 16
HD = 64
NCORES = 8
HPC = 8          # heads per core
DG = HPC * HD    # 512 local head channels
P = 128          # partition tile
TC = T // 512    # 4 q-chunks of 512
KT = T // P      # 16 k-tiles
DT = D // P      # 8 contraction tiles for qkv
NEG = -1.0e5


def _build(has_bqkv: bool, has_bproj: bool):
    import concourse.bass as bass
    import concourse.bacc as bacc
    import concourse.mybir as mybir
    import concourse.tile as tile
    from contextlib import ExitStack

    f32 = mybir.dt.float32
    bf16 = mybir.dt.bfloat16
    EXP = mybir.ActivationFunctionType.Exp

    nc = bacc.Bacc(num_devices=NCORES)

    xT = nc.declare_dram_parameter("xT", [D, T], bf16, isOutput=False)
    wq = nc.declare_dram_parameter("wq", [D, DG], bf16, isOutput=False)
    wk = nc.declare_dram_parameter("wk", [D, DG], bf16, isOutput=False)
    wv = nc.declare_dram_parameter("wv", [D, DG], bf16, isOutput=False)
    wp = nc.declare_dram_parameter("wp", [DG, D], bf16, isOutput=False)
    maskneg = nc.declare_dram_parameter("maskneg", [P, P], f32, isOutput=False)
    if has_bqkv:
        bq = nc.declare_dram_parameter("bq", [1, DG], f32, isOutput=False)
        bk = nc.declare_dram_parameter("bk", [1, DG], f32, isOutput=False)
        bv = nc.declare_dram_parameter("bv", [1, DG], f32, isOutput=False)
    if has_bproj:
        bp = nc.declare_dram_parameter("bp", [1, D], f32, isOutput=False)
    # output rows: 4 chunks of 256 (this core's half of each 512 q-chunk)
    out_ext = nc.declare_dram_parameter("out", [T // 2, D], bf16, isOutput=True)

    with tile.TileContext(nc) as tc, ExitStack() as ctx:
        persist = ctx.enter_context(tc.tile_pool(name="persist", bufs=1))
        mmpool = ctx.enter_context(tc.tile_pool(name="mmpool", bufs=2, space="PSUM"))
        spool = ctx.enter_context(tc.tile_pool(name="spool", bufs=2, space="PSUM"))
        ypool = ctx.enter_context(tc.tile_pool(name="ypool", bufs=2, space="PSUM"))
        ptpool = ctx.enter_context(tc.tile_pool(name="ptpool", bufs=8))
        popool = ctx.enter_context(tc.tile_pool(name="popool", bufs=4))
        stagepool = ctx.enter_context(tc.tile_pool(name="stagepool", bufs=8))
        recpool = ctx.enter_context(tc.tile_pool(name="recpool", bufs=4))
        dram = ctx.enter_context(tc.tile_pool(name="dram", bufs=1, space="DRAM"))

        def pt_tiles(name, n, cols, dt=bf16):
            return [persist.tile([P, cols], dt, tag=f"{name}{i}",
                                 name=f"{name}{i}")
                    for i in range(n)]

        xT_sb = pt_tiles("xt", DT, T)            # 8 x [128, 2048]
        wq_sb = pt_tiles("wq", DT, DG)           # 8 x [128, 512]
        wk_sb = pt_tiles("wk", DT, DG)
        wv_sb = pt_tiles("wv", DT, DG)
        wp_sb = pt_tiles("wp", DG // P, D)       # 4 x [128, 1024]
        qt_sb = pt_tiles("qt", DG // P, T)       # 4 x [128, 2048]
        kt_sb = pt_tiles("kt", DG // P, T)
        v_sb = pt_tiles("vv", KT, HPC * P)       # 16 x [128, 1024] (V|ones)
        yt_sb = pt_tiles("yt", DG // P, T)       # 4 x [128, 2048]
        mneg_sb = persist.tile([P, P], f32, tag="mneg")

        # DMA order: wq/wk + x cols 0:512 first (what the first q0/k0
        # matmuls need), then the rest of x, wv, wp.
        for i in range(DT):
            nc.sync.dma_start(out=wq_sb[i], in_=wq[i * P:(i + 1) * P, :])
            nc.sync.dma_start(out=wk_sb[i], in_=wk[i * P:(i + 1) * P, :])
            nc.sync.dma_start(out=xT_sb[i][:, 0:512],
                              in_=xT[i * P:(i + 1) * P, 0:512])
        nc.sync.dma_start(out=mneg_sb, in_=maskneg[:, :])
        # wv next (chunk 0's v tiles read x cols 0:512 + all of wv); the
        # x tails are only needed from chunk 1 on
        for i in range(DT):
            nc.sync.dma_start(out=wv_sb[i], in_=wv[i * P:(i + 1) * P, :])
        for i in range(DT):
            nc.sync.dma_start(out=xT_sb[i][:, 512:T],
                              in_=xT[i * P:(i + 1) * P, 512:T])
        for i in range(DG // P):
            nc.sync.dma_start(out=wp_sb[i], in_=wp[i * P:(i + 1) * P, :])
        # ones blocks of v_sb (cols h*128+64 .. h*128+127): memset whole
        # tile to 1.0 on the idle gpsimd engine; V overwrites cols 0:64.
        for t in range(KT):
            nc.gpsimd.memset(v_sb[t], 1.0)

        if has_bqkv or has_bproj:
            ones_sb = persist.tile([1, P], bf16, tag="ones")
            nc.vector.memset(ones_sb, 1.0)
            if has_bqkv:
                bq_sb = persist.tile([1, DG], bf16, tag="bq")
                bk_sb = persist.tile([1, DG], bf16, tag="bk")
                bv_sb = persist.tile([1, DG], bf16, tag="bv")
                bq_f = persist.tile([1, DG], f32, tag="bqf")
                bk_f = persist.tile([1, DG], f32, tag="bkf")
                bv_f = persist.tile([1, DG], f32, tag="bvf")
                nc.sync.dma_start(out=bq_f, in_=bq[:, :])
                nc.sync.dma_start(out=bk_f, in_=bk[:, :])
                nc.sync.dma_start(out=bv_f, in_=bv[:, :])
                nc.vector.tensor_copy(bq_sb, bq_f)
                nc.vector.tensor_copy(bk_sb, bk_f)
                nc.vector.tensor_copy(bv_sb, bv_f)
            if has_bproj:
                bp_sb = persist.tile([1, D], bf16, tag="bp")
                bp_f = persist.tile([1, D], f32, tag="bpf")
                nc.sync.dma_start(out=bp_f, in_=bp[:, :])
                nc.vector.tensor_copy(bp_sb, bp_f)

        # ---- QKV projection groups, as generators yielding after every
        # matmul so they can be woven one-MM-at-a-time into the ACT-paced
        # attention stream (keeps PE dense and HAM warm) ----
        def qkv_ft_gen(which, w_sb, o_sb, f, c):
            ps = mmpool.tile([P, 512], f32, tag="mm", name=f"qkv{which}{f}_{c}")
            for k in range(DT):
                last = k == DT - 1
                nc.tensor.matmul(
                    ps,
                    lhsT=w_sb[k][:, f * P:(f + 1) * P],
                    rhs=xT_sb[k][:, c * 512:(c + 1) * 512],
                    start=(k == 0),
                    stop=(last and not has_bqkv),
                )
                if not last:
                    yield
            if has_bqkv:
                bsl = (bq_sb if which == "q" else bk_sb)
                nc.tensor.matmul(
                    ps,
                    lhsT=bsl[0:1, f * P:(f + 1) * P],
                    rhs=ones_sb[0:1, 0:1].to_broadcast((1, 512)),
                    start=False, stop=True,
                )
            nc.vector.tensor_copy(o_sb[f][:, c * 512:(c + 1) * 512], ps)
            yield

        def v_tile_gen(t):
            ps = mmpool.tile([P, 512], f32, tag="mm", name=f"vt{t}")
            for k in range(DT):
                last = k == DT - 1
                nc.tensor.matmul(
                    ps,
                    lhsT=xT_sb[k][:, t * P:(t + 1) * P],
                    rhs=wv_sb[k],
                    start=(k == 0),
                    stop=(last and not has_bqkv),
                )
                if not last:
                    yield
            if has_bqkv:
                nc.tensor.matmul(
                    ps, lhsT=ones_sb[0:1, 0:P], rhs=bv_sb,
                    start=False, stop=True,
                )
            vg = v_sb[t].rearrange("p (h x) -> p h x", h=HPC)
            nc.vector.tensor_copy(
                vg[:, :, 0:HD],
                ps.rearrange("p (h x) -> p h x", h=HPC),
            )
            yield

        def qkv_chunk_gens(c):
            # order: q0,k0,v0..v3 first (everything attention(c, hp=0)
            # needs), then q1,k1,..,q3,k3 drained per head-pair
            g = [qkv_ft_gen("q", wq_sb, qt_sb, 0, c),
                 qkv_ft_gen("k", wk_sb, kt_sb, 0, c)]
            for t in range(4 * c, 4 * c + 4):
                g.append(v_tile_gen(t))
            for f in range(1, DG // P):
                g.append(qkv_ft_gen("q", wq_sb, qt_sb, f, c))
                g.append(qkv_ft_gen("k", wk_sb, kt_sb, f, c))
            return g

        from collections import deque
        fill_q = deque()          # (chunk_id, gen)
        done_cnt = {}             # chunk_id -> completed gens

        def pull(n):
            while n > 0 and fill_q:
                cid, g = fill_q[0]
                try:
                    next(g)
                    n -= 1
                except StopIteration:
                    done_cnt[cid] = done_cnt.get(cid, 0) + 1
                    fill_q.popleft()

        def drain_until(cid, cnt):
            # run the queue until `cnt` gens of chunk `cid` have completed
            # (gens are queued in FIFO order, so everything ahead of them
            # completes too)
            while done_cnt.get(cid, 0) < cnt and fill_q:
                c0, g = fill_q[0]
                try:
                    next(g)
                except StopIteration:
                    done_cnt[c0] = done_cnt.get(c0, 0) + 1
                    fill_q.popleft()

        def drain_fill():
            while fill_q:
                cid, g = fill_q[0]
                try:
                    next(g)
                except StopIteration:
                    done_cnt[cid] = done_cnt.get(cid, 0) + 1
                    fill_q.popleft()

        # ---- attention for one head-pair + q-chunk ----
        # Heads 2i and 2i+1 live in rows 0-63 / 64-127 of qt_sb[i]/kt_sb[i];
        # their S^T matmuls are emitted back-to-back so the PE packs them
        # into disjoint row-strips of the array (tile_position from base
        # partition) and overlaps the weight loads. One wide exp covers both.
        def attn_pair_chunk(hp, qc):
            qt = qt_sb[hp]                   # [128, 2048]: h0 rows 0-63, h1 64-127
            kt = kt_sb[hp]
            h0, h1 = 2 * hp, 2 * hp + 1
            yps0 = ypool.tile([P, 512], f32, tag="y", name=f"y0_{hp}_{qc}")
            yps1 = ypool.tile([P, 512], f32, tag="y", name=f"y1_{hp}_{qc}")
            nj = 4 * qc + 4
            for j in range(nj):
                off = (j - 4 * qc) * P    # <=0 for out-of-chunk k-tiles
                o = max(0, off)
                ncols = 512 - o
                q0 = qc * 512 + o
                sps = spool.tile([P, 1024], f32, tag="s", name=f"s{hp}_{qc}_{j}")
                nc.tensor.matmul(
                    sps[:, o:o + ncols],
                    lhsT=kt[0:HD, j * P:(j + 1) * P],
                    rhs=qt[0:HD, q0:q0 + ncols],
                    start=True, stop=True,
                )
                nc.tensor.matmul(
                    sps[:, 512 + o:512 + o + ncols],
                    lhsT=kt[HD:P, j * P:(j + 1) * P],
                    rhs=qt[HD:P, q0:q0 + ncols],
                    start=True, stop=True,
                )
                if off >= 0:
                    # diagonal block of both heads: mask q < k before exp
                    sg = sps.rearrange("p (g x) -> p g x", g=2)[:, :, o:o + P]
                    nc.vector.tensor_add(
                        sg, sg, mneg_sb[:, None, :].to_broadcast((P, 2, P)))
                pt = ptpool.tile([P, 1024], bf16, tag="pt",
                                 name=f"pt{hp}_{qc}_{j}")
                nc.scalar.activation(
                    pt[:, o:1024], sps[:, o:1024], EXP, scale=0.125)
                nc.tensor.matmul(
                    yps0[:, o:512],
                    lhsT=v_sb[j][:, h0 * P:(h0 + 1) * P],
                    rhs=pt[:, o:o + ncols],
                    start=(j == 0), stop=(j == nj - 1),
                )
                nc.tensor.matmul(
                    yps1[:, o:512],
                    lhsT=v_sb[j][:, h1 * P:(h1 + 1) * P],
                    rhs=pt[:, 512 + o:512 + o + ncols],
                    start=(j == 0), stop=(j == nj - 1),
                )
                # late chunks have fewer fills per j: spread them so the
                # queue lasts the whole chunk (a fill between PV(j) and
                # S(j+1) is also what hides the scores weight loads)
                pull(2 if qc < 2 else 1)
            # rows 0-63: unnormalized Y^T; rows 64-127: rowsum replicated
            for hi, yps in ((h0, yps0), (h1, yps1)):
                ti, ro = hi // 2, (hi % 2) * HD
                rec = recpool.tile([HD, 512], f32, tag="rec",
                                   name=f"rec{hi}_{qc}")
                rsum = recpool.tile([HD, 512], f32, tag="rsum",
                                    name=f"rsum{hi}_{qc}")
                nc.vector.tensor_copy(rsum, yps[HD:2 * HD, :])
                nc.vector.reciprocal_approx_fast(rec, rsum)
                nc.vector.tensor_mul(
                    yt_sb[ti][ro:ro + HD, qc * 512:(qc + 1) * 512],
                    yps[0:HD, :], rec)

        # ---- partial projection + chunked 2-way ReduceScatter ----
        groups = [[2 * b, 2 * b + 1] for b in range(B)]

        # RS part tables: (pp_row_start, pp_rows). Mid-kernel chunks use 2
        # parts (overlap); the last chunk uses ONE (RS fixed cost ~10us
        # dominates, and the ops serialize on one CC stream).
        def parts(qc):
            if qc == TC - 1:
                return [(0, 512)]
            return [(0, 256), (256, 256)]

        # partials and the 2-way reduce run in bf16 (halves RS bytes; the
        # 2-term sum costs ~0.4% relative on the partials, well under gate)
        pp_t = {(qc, pi): dram.tile([pr[1], D], bf16,
                                    tag=f"pp{qc}_{pi}", name=f"pp{qc}_{pi}")
                for qc in range(TC) for pi, pr in enumerate(parts(qc))}
        rs_t = {(qc, pi): dram.tile([pr[1] // 2, D], bf16,
                                    tag=f"rs{qc}_{pi}", name=f"rs{qc}_{pi}")
                for qc in range(TC) for pi, pr in enumerate(parts(qc))}

        def rs_chunk(qc, pi):
            # 2-way ReduceScatter (collectives cannot write IO tensors, so
            # reduce into an internal tile and DMA to the bf16 output; both
            # sides are bf16 so it is a pure copy, no DVE involved)
            rstart, rrows = parts(qc)[pi]
            half = rrows // 2
            r0 = qc * 256 + rstart // 2
            nc.gpsimd.collective_compute(
                "ReduceScatter",
                mybir.AluOpType.add,
                replica_groups=groups,
                ins=[pp_t[(qc, pi)].opt()],
                outs=[rs_t[(qc, pi)].opt()],
            )
            nc.sync.dma_start(out=out_ext[r0:r0 + half, :],
                              in_=rs_t[(qc, pi)])

        def proj_group_gen(qc, t, chn):
            tl = t - 4 * qc
            pi = 0 if qc == TC - 1 else tl // 2
            rbase = parts(qc)[pi][0]
            ro = tl * P - rbase
            ps = mmpool.tile([P, 512], f32, tag="mm", name=f"pj{t}_{chn}")
            for k4 in range(DG // P):
                last = k4 == DG // P - 1
                nc.tensor.matmul(
                    ps,
                    lhsT=yt_sb[k4][:, t * P:(t + 1) * P],
                    rhs=wp_sb[k4][:, chn * 512:(chn + 1) * 512],
                    start=(k4 == 0),
                    stop=(last and not has_bproj),
                )
                if not last:
                    yield
            if has_bproj:
                nc.tensor.matmul(
                    ps,
                    lhsT=ones_sb[0:1, 0:P],
                    rhs=bp_sb[0:1, chn * 512:(chn + 1) * 512],
                    start=False, stop=True,
                )
            po = popool.tile([P, 512], bf16, tag="po", name=f"po{t}_{chn}")
            nc.vector.tensor_copy(po, ps)
            nc.sync.dma_start(
                out=pp_t[(qc, pi)][ro:ro + P, chn * 512:(chn + 1) * 512],
                in_=po)
            # after the last group of an RS part, fire its collective
            if tl * P + P == rbase + parts(qc)[pi][1] and chn == 1:
                rs_chunk(qc, pi)
            yield

        def proj_rs_gens(qc):
            return [proj_group_gen(qc, t, chn)
                    for t in range(4 * qc, 4 * qc + 4)
                    for chn in range(D // 512)]

        # Last chunk: split the proj contraction so only half of it sits
        # on the post-attention critical path. k4 in {0,1} (head-pairs
        # 0/1's yt tiles) run during hp2/hp3's attention and stage to
        # SBUF; k4 in {2,3} + add + DMA + the single RS run at the end.
        stage_sb = {}

        def proj_first_gen(t, chn):
            ps = mmpool.tile([P, 512], f32, tag="mm", name=f"pjA{t}_{chn}")
            for k4 in (0, 1):
                nc.tensor.matmul(
                    ps,
                    lhsT=yt_sb[k4][:, t * P:(t + 1) * P],
                    rhs=wp_sb[k4][:, chn * 512:(chn + 1) * 512],
                    start=(k4 == 0), stop=(k4 == 1),
                )
                yield
            st = stagepool.tile([P, 512], f32, tag="stg", name=f"st{t}_{chn}")
            # stage on ScalarE: DVE is busy with the last normalizes, and
            # the final adds must not queue behind them
            nc.scalar.copy(st, ps)
            stage_sb[(t, chn)] = st
            yield

        def proj_second_gen(t, chn):
            qc = TC - 1
            tl = t - 4 * qc
            ps = mmpool.tile([P, 512], f32, tag="mm", name=f"pjB{t}_{chn}")
            for k4 in (2, 3):
                nc.tensor.matmul(
                    ps,
                    lhsT=yt_sb[k4][:, t * P:(t + 1) * P],
                    rhs=wp_sb[k4][:, chn * 512:(chn + 1) * 512],
                    start=(k4 == 2),
                    stop=(k4 == 3 and not has_bproj),
                )
            if has_bproj:
                nc.tensor.matmul(
                    ps,
                    lhsT=ones_sb[0:1, 0:P],
                    rhs=bp_sb[0:1, chn * 512:(chn + 1) * 512],
                    start=False, stop=True,
                )
            po = popool.tile([P, 512], bf16, tag="po", name=f"po{t}_{chn}")
            nc.vector.tensor_add(po, ps, stage_sb[(t, chn)])
            nc.sync.dma_start(
                out=pp_t[(qc, 0)][tl * P:tl * P + P,
                                  chn * 512:(chn + 1) * 512],
                in_=po)
            if tl == 3 and chn == 1:
                rs_chunk(qc, 0)

        # ---- interleaved emission ----
        # Chunk 0's q0/k0/v tiles are emitted eagerly; everything else
        # weaves through the attention stream. Before attention(hp, c) we
        # only require chunk c's first 6+2*hp qkv gens (q0,k0,v0..3,
        # then qf,kf per pair) to be complete — no full barrier.
        c0 = qkv_chunk_gens(0)
        for g in c0[0:6]:
            deque(g, maxlen=0)
        done_cnt[0] = 6
        fill_q.extend((0, g) for g in c0[6:])
        for c in range(TC):
            if c + 1 < TC:
                fill_q.extend((c + 1, g) for g in qkv_chunk_gens(c + 1))
            if c >= 1:
                fill_q.extend((100 + c - 1, g) for g in proj_rs_gens(c - 1))
            for hp in range(HPC // 2):
                drain_until(c, 6 + 2 * hp)
                attn_pair_chunk(hp, c)
                pull(4 if c < 2 else 2)
                if c == TC - 1 and hp == 1:
                    fill_q.extend((200, proj_first_gen(t, chn))
                                  for t in range(4 * c, 4 * c + 4)
                                  for chn in range(D // 512))
            if c + 1 < TC:
                drain_until(c + 1, 6)
        drain_fill()
        for t in range(4 * (TC - 1), 4 * (TC - 1) + 4):
            for chn in range(D // 512):
                proj_second_gen(t, chn)

    return nc


def kernel(x, W_qkv, b_qkv, W_proj, b_proj):
    import sys
    if "/opt/trn_rl_repo" not in sys.path:
        sys.path.insert(0, "/opt/trn_rl_repo")
    from concourse.bass_utils import run_bass_kernel_spmd

    x = np.asarray(x, dtype=np.float32)
    W_qkv = np.asarray(W_qkv, dtype=np.float32)
    b_qkv = np.asarray(b_qkv, dtype=np.float32)
    W_proj = np.asarray(W_proj, dtype=np.float32)
    b_proj = np.asarray(b_proj, dtype=np.float32)

    has_bqkv = bool(np.any(b_qkv))
    has_bproj = bool(np.any(b_proj))
    nc = _build(has_bqkv, has_bproj)
    nc.finalize()

    bf = ml_dtypes.bfloat16
    # causal mask for the S^T diagonal block: S^T[k, q] valid iff q >= k
    mneg = np.where(
        np.arange(P)[None, :] >= np.arange(P)[:, None], 0.0, NEG
    ).astype(np.float32)

    wq_g = [np.ascontiguousarray(W_qkv[:, g * DG:(g + 1) * DG]).astype(bf)
            for g in range(2)]
    wk_g = [np.ascontiguousarray(W_qkv[:, D + g * DG:D + (g + 1) * DG]).astype(bf)
            for g in range(2)]
    wv_g = [np.ascontiguousarray(W_qkv[:, 2 * D + g * DG:2 * D + (g + 1) * DG]).astype(bf)
            for g in range(2)]
    wp_g = [np.ascontiguousarray(W_proj[g * DG:(g + 1) * DG, :]).astype(bf)
            for g in range(2)]

    in_maps = []
    for c in range(NCORES):
        b, g = c // 2, c % 2
        m = {
            "xT": np.ascontiguousarray(x[b].T).astype(bf),
            "wq": wq_g[g],
            "wk": wk_g[g],
            "wv": wv_g[g],
            "wp": wp_g[g],
            "maskneg": mneg,
        }
        if has_bqkv:
            m["bq"] = b_qkv[None, g * DG:(g + 1) * DG].copy()
            m["bk"] = b_qkv[None, D + g * DG:D + (g + 1) * DG].copy()
            m["bv"] = b_qkv[None, 2 * D + g * DG:2 * D + (g + 1) * DG].copy()
        if has_bproj:
            # bias must be added once per pair: zero it on the odd core
            m["bp"] = b_proj[None, :].copy() if g == 0 else np.zeros(
                (1, D), np.float32)
        in_maps.append(m)

    res = run_bass_kernel_spmd(nc, in_maps, core_ids=list(range(NCORES)))
    out = np.empty((B, T, D), dtype=np.float32)
    part_tabs = {qc: ([(0, 512)] if qc == TC - 1 else [(0, 256), (256, 256)])
                 for qc in range(TC)}
    for c in range(NCORES):
        b, g = c // 2, c % 2
        o = res.results[c]["out"].astype(np.float32)   # [1024, 1024] bf16
        for qc in range(TC):
            for rstart, rrows in part_tabs[qc]:
                half = rrows // 2
                src = qc * 256 + rstart // 2
                dst = qc * 512 + rstart + g * half
                out[b, dst:dst + half, :] = o[src:src + half, :]
    return out
